# revision 22
# baseline (speedup 1.0000x reference)
"""Block-causal attention (BlockDiffusionDecoder) on 8 TRN2 NeuronCores.

Reference computes, per (b, h):
    S = (Q K^T) / 8, masked so query block i (64 rows) attends key blocks <= i,
    O = softmax(S) V,   shapes [2, 16, 2048, 64] f32.

Sharding: batch*heads (32) split across 8 cores, 4 heads per core, no comm.

Per-core algorithm (all matmuls bf16, fp32 accumulate):
  - Q,K are staged to DRAM bf16 [2048, 128] (two heads side by side) and
    transposed via the DMA xbar into SBUF [128, 2048]: partitions 0:64 hold
    head-even Q^T/K^T, 64:128 head-odd. The two heads of a pair are
    processed interleaved: their QK^T matmuls are 64-contraction row-tiled
    ops on disjoint partition halves, so the PE runs them concurrently.
  - Scores are computed transposed (S^T tile [128 k, 256 q]): stationary
    K^T_j [64, 128], moving Q^T [64, 256]. Both heads' scores for a
    3-k-tile group share one PSUM tile [128, 1536] -> one exp call.
  - softmax exp without max-subtraction (|scores/8| <= ~6 so fp32/bf16 exp
    is safe), via ScalarE from PSUM, split with VectorE: the leading
    k-groups of the later q-items run as Schraudolph fast-exp on VectorE
    (int16 tensor_scalar whose bits are the bf16 P^T) per OFFL, balancing
    ScalarE ~48us/rep against VectorE ~46us/rep.
  - Block mask applied by zeroing P^T sub-blocks (GpSimd memset, keeping
    the DVE FIFO clear) or by skipping the fully-masked half-block in the
    matmuls / PV reads.
  - PV: stationary V'_j [128 k, 65] (V plus a ones column -> row sums land
    in output row 64), moving P^T, accumulating O^T [65, 256] in PSUM.
  - O^T -> O via DMA xbar transpose (bf16), then normalize rows by the
    reciprocal of the softmax sum on VectorE and DMA out as f32.
  - Issue order is a flat software pipeline ACROSS reps (q-items emitted
    big-exp-first within each head pair; staging prefetched a full pair
    ahead; head post-processing split so the ot-transpose wait never
    blocks the DVE FIFO). Input DMA streams are need-order chained
    per-kind only (staging / transposes / v loads), so a WAR-blocked
    transpose cannot back-pressure unrelated streams.
"""

import numpy as np

B, H, S, D = 2, 16, 2048, 64
N_CORES = 8
HPC = (B * H) // N_CORES  # heads per core = 4
NP = HPC // 2  # head pairs per core = 2
QP = S // 256  # q-pairs (two 128-row q-tiles per step) = 8
GW = 1536  # PSUM score-group width: 3 k-tiles x 256 q, both heads

# Schraudolph exp2 constants, int16/bf16 form:
# exp(0.125*s) ~= bitcast_bf16(i16(s*SCH_A + SCH_B))
SCH_A = 128.0 * 0.125 * 1.4426950408889634
SCH_B = 127.0 * 128.0 - 334700.0 / 65536.0

_CACHE = {}

# Per-item count of LEADING k-groups whose exp runs as Schraudolph
# fast-exp on VectorE (int16 tensor_scalar -> bf16 bit pattern) instead of
# exact exp on ScalarE. Offloading balances the two engines. The QK
# matmuls of ScalarE's (non-offloaded) groups are emitted FIRST within
# each item so the exact exps chase the head of the QK stream; the
# offloaded groups' scores are computed at the end of the item and only
# needed by the one-item-lagged PV. Accuracy degrades with coverage
# (Schraudolph is a +-2.9% sawtooth).
import os as _os
_OFFL_DEF = "0,0,0,0,1,2,2,3"
OFFL = [int(x) for x in _os.environ.get("OFFL", _OFFL_DEF).split(",")]


def _build(reps=1):
    import concourse.bass as bass
    import concourse.mybir as mybir
    from concourse import bacc
    from concourse.bass import ts
    from concourse.tile import TileContext
    from concourse.tile_rust import add_dep_helper

    f32 = mybir.dt.float32
    bf16 = mybir.dt.bfloat16

    nc = bacc.Bacc("TRN2", target_bir_lowering=False, debug=False,
                   num_devices=N_CORES)
    q = nc.declare_dram_parameter("q", [HPC, S, D], f32, isOutput=False)
    k = nc.declare_dram_parameter("k", [HPC, S, D], f32, isOutput=False)
    v = nc.declare_dram_parameter("v", [HPC, S, D], f32, isOutput=False)
    out = nc.declare_dram_parameter("out", [HPC, S, D], f32, isOutput=True)

    def off(j, e):  # free offset of k-tile j (head-half e) inside pT
        return (j // 3) * GW + e * 768 + (j % 3) * 256

    chain_prev = {}

    def chained(bi, kind="in"):
        # Per-kind need-order chains: keeps each DMA stream (qk staging /
        # qkT transposes / v loads) issue-ordered without serializing the
        # streams against each other (a cross-kind chain lets a WAR-blocked
        # transpose back-pressure unrelated staging and, through the Pool
        # FIFO, the mask memsets and PV).
        if chain_prev.get(kind) is not None:
            add_dep_helper(bi.ins, chain_prev[kind].ins, sync=False,
                           reason="dma need-order")
        chain_prev[kind] = bi
        return bi

    with TileContext(nc) as tc:
        with (
            tc.tile_pool(name="dram_stage", bufs=2, space="DRAM") as dpool,
            tc.tile_pool(name="qkT", bufs=3) as qkT_pool,
            tc.tile_pool(name="vsb", bufs=4) as v_pool,
            tc.tile_pool(name="pT", bufs=3) as pT_pool,
            tc.tile_pool(name="osb", bufs=4) as o_pool,
            tc.tile_pool(name="ps", bufs=2, space="PSUM") as ps_pool,
            tc.tile_pool(name="oT", bufs=2, space="PSUM") as oT_pool,
        ):
            qT = {}
            kT = {}
            v_sb = {}
            oT_sb = {}
            sqk_st = {}
            ot_sb = {}
            rec_sb = {}

            def emit_pair_stage_qk(i):
                p = i % NP
                sqk = dpool.tile([2 * S, 128], bf16, name=f"sqk{i}", tag="sqk")
                chained(nc.gpsimd.dma_start(
                    out=sqk[0:S, :].rearrange("s (e d) -> s e d", e=2),
                    in_=q[2 * p:2 * p + 2].rearrange("e s d -> s e d"),
                ), "qk")
                chained(nc.gpsimd.dma_start(
                    out=sqk[S:2 * S, :].rearrange("s (e d) -> s e d", e=2),
                    in_=k[2 * p:2 * p + 2].rearrange("e s d -> s e d"),
                ), "qk")
                sqk_st[i] = sqk

            def emit_pair_stage_tr(i):
                qkT_t = qkT_pool.tile([128, 2 * S], bf16, name=f"qkT{i}",
                                      tag="qkT")
                chained(nc.sync.dma_start_transpose(out=qkT_t[:],
                                                    in_=sqk_st[i][:]), "tr")
                qT[i] = qkT_t[:, 0:S]
                kT[i] = qkT_t[:, S:2 * S]

            def emit_pair_pre(i):
                p = i % NP
                vs = v_pool.tile([128, 2, 16, 65], bf16, name=f"v{i}", tag="v")
                chained(nc.gpsimd.dma_start(
                    out=vs[:, :, :, 0:64],
                    in_=v[2 * p:2 * p + 2].rearrange("e (n p) d -> p e n d",
                                                     p=128),
                ), "v")
                nc.gpsimd.memset(vs[:, :, :, 64], 1.0)
                v_sb[i] = vs
                oT_sb[i] = o_pool.tile([128, 2 * S], bf16, name=f"o{i}",
                                       tag="osb")

            def emit_scores(i, t):
                """Both heads of pair i, q-pair t. Returns a pread(j, e)
                accessor over the P^T storage (pT tile + optional int view)."""
                jmax = 2 * t + 1
                ngroups = (jmax + 3) // 3
                pT = pT_pool.tile([128, 6 * GW], bf16, name=f"pT_{i}_{t}",
                                  tag="pT")
                noff = OFFL[t] if t < len(OFFL) else 0
                for g in range(ngroups):
                    jn = min(3, jmax + 1 - 3 * g)
                    ps = ps_pool.tile([128, GW], mybir.dt.float32,
                                      name=f"ps_{i}_{t}_{g}", tag="ps")
                    for jj in range(jn):
                        j = 3 * g + jj
                        half = 128 if j == jmax else 0
                        for e in range(2):  # interleave heads: row-tile pair
                            nc.tensor.matmul(
                                ps[:, e * 768 + jj * 256 + half:
                                   e * 768 + (jj + 1) * 256],
                                kT[i][64 * e:64 * e + 64, ts(j, 128)],
                                qT[i][64 * e:64 * e + 64,
                                      t * 256 + half:(t + 1) * 256],
                                start=True, stop=True,
                            )
                    if g < noff:
                        # Schraudolph fast exp on VectorE: one tensor_scalar
                        # with int16 output whose bits are the bf16 P^T.
                        nc.vector.tensor_scalar(
                            pT[:, g * GW:(g + 1) * GW].bitcast(
                                mybir.dt.int16),
                            ps[:], SCH_A, SCH_B,
                            mybir.AluOpType.mult, mybir.AluOpType.add)
                    elif jn == 3:
                        nc.scalar.activation(
                            pT[:, g * GW:(g + 1) * GW], ps[:],
                            mybir.ActivationFunctionType.Exp, scale=0.125)
                    else:
                        w = jn * 256
                        nc.scalar.activation(
                            pT[:, g * GW:(g + 1) * GW].rearrange(
                                "q (e r) -> q e r", e=2)[:, :, :w],
                            ps[:].rearrange("q (e r) -> q e r", e=2)[:, :, :w],
                            mybir.ActivationFunctionType.Exp, scale=0.125)

                def pread(j, e, c0=0, c1=256, r0=0):
                    o = off(j, e)
                    return pT[r0:128, o + c0:o + c1]

                for e in range(2):
                    nc.gpsimd.memset(pread(2 * t, e, 0, 64, 64), 0.0)
                    nc.gpsimd.memset(pread(2 * t + 1, e, 128, 192, 64), 0.0)
                return pread

            def emit_pv(i, t, pread):
                # Both heads' O^T accumulate into ONE PSUM bank [65, 512]
                # (e0 cols 0:256, e1 cols 256:512): the 2-buffer oT ring then
                # spans two ITEMS, so the copy may lag a full item without
                # stalling PV, and one DVE copy moves both heads.
                jmax = 2 * t + 1
                oT = oT_pool.tile([65, 512], mybir.dt.float32,
                                  name=f"oT_{i}_{t}", tag="oT")
                for e in range(2):
                    for j in range(jmax):
                        nc.tensor.matmul(
                            oT[:, e * 256:e * 256 + 256],
                            v_sb[i][:, e, j, :], pread(j, e),
                            start=(j == 0), stop=False,
                        )
                    # k-tile jmax: q-tile 2t is fully below it -> only the
                    # right 128 columns (q-tile 2t+1) see it.
                    nc.tensor.matmul(
                        oT[:, e * 256 + 128:e * 256 + 256],
                        v_sb[i][:, e, jmax, :],
                        pread(jmax, e, 128, 256),
                        start=False, stop=True,
                    )
                nc.vector.tensor_copy(
                    oT_sb[i][0:65].rearrange("p (e s) -> p e s", e=2)
                    [:, :, t * 256:(t + 1) * 256],
                    oT[:].rearrange("p (e c) -> p e c", e=2))

            def emit_head_dma(i, e):
                ot = o_pool.tile([128, 16, 80], bf16, name=f"ot{i}_{e}",
                                 tag="ot")
                nc.sync.dma_start_transpose(
                    out=ot[:], in_=oT_sb[i][0:80, e * S:(e + 1) * S])
                ot_sb[(i, e)] = ot

            def emit_head_norm(i, e):
                h = 2 * (i % NP) + e
                ot = ot_sb[(i, e)]
                rec = o_pool.tile([128, 16], mybir.dt.float32,
                                  name=f"rec{i}_{e}", tag="rec")
                nc.vector.reciprocal(rec[:], ot[:, :, 64])
                of = o_pool.tile([128, 16, 64], mybir.dt.float32,
                                 name=f"of{i}_{e}", tag="of")
                for n in range(16):
                    nc.vector.tensor_scalar_mul(
                        of[:, n, :], ot[:, n, 0:64], rec[:, n:n + 1])
                nc.sync.dma_start(
                    out=out[h].rearrange("(n p) d -> p n d", p=128), in_=of[:])

            # ---- software-pipelined issue order (flat across reps) ----
            pending = None
            posted = None
            npairs = reps * NP
            emit_pair_stage_qk(0)
            emit_pair_stage_tr(0)
            emit_pair_pre(0)
            if npairs > 1:
                emit_pair_stage_qk(1)
                emit_pair_stage_tr(1)
                emit_pair_pre(1)
            for i in range(npairs):
                # Big-exp items first: the triangular exp backlog (t=7 has 6
                # groups) lands where the pair's dense QK gives ScalarE
                # runway; the boundary item (t=0, one group) drains fast so
                # the next pair's QK isn't starved behind the PSUM ring.
                for idx, t in enumerate(reversed(range(QP))):
                    pread = emit_scores(i, t)
                    if idx == 0 and i + 2 < npairs:
                        emit_pair_stage_qk(i + 2)
                    if idx == 3 and i + 2 < npairs:
                        emit_pair_stage_tr(i + 2)
                    if idx == 5 and i + 2 < npairs:
                        emit_pair_pre(i + 2)
                    if pending is not None:
                        pp, pt, ppread = pending
                        emit_pv(pp, pt, ppread)
                        if pt == 0:
                            emit_head_dma(pp, 0)
                            emit_head_dma(pp, 1)
                            posted = pp
                    if idx == 2 and posted is not None:
                        emit_head_norm(posted, 0)
                        emit_head_norm(posted, 1)
                        posted = None
                    pending = (i, t, pread)
            pp, pt, ppread = pending
            emit_pv(pp, pt, ppread)
            emit_head_dma(pp, 0)
            emit_head_dma(pp, 1)
            emit_head_norm(pp, 0)
            emit_head_norm(pp, 1)

    nc.compile()
    return nc


def _get_nc():
    if "nc" not in _CACHE:
        _CACHE["nc"] = _build()
    return _CACHE["nc"]


def kernel(q, k, v):
    from concourse.bass_utils import run_bass_kernel_spmd

    nc = _get_nc()
    qf = np.ascontiguousarray(q, dtype=np.float32).reshape(B * H, S, D)
    kf = np.ascontiguousarray(k, dtype=np.float32).reshape(B * H, S, D)
    vf = np.ascontiguousarray(v, dtype=np.float32).reshape(B * H, S, D)
    in_maps = [
        {
            "q": qf[c * HPC:(c + 1) * HPC],
            "k": kf[c * HPC:(c + 1) * HPC],
            "v": vf[c * HPC:(c + 1) * HPC],
        }
        for c in range(N_CORES)
    ]
    res = run_bass_kernel_spmd(nc, in_maps, core_ids=list(range(N_CORES)))
    full = np.concatenate([res.results[c]["out"] for c in range(N_CORES)], axis=0)
    return full.reshape(B, H, S, D).astype(np.float32)

